# revision 7
# baseline (speedup 1.0000x reference)
"""Causal single-head attention (B=4, T=4096, D=1024) on 8 trn2 NeuronCores.

Sharding: 2 cores per batch element, split by key-block PARITY (flash-style):
  core = 2*b + p ; p in {0,1}
  Each core computes, for ALL 4096 queries of batch b, the partial
  (unnormalized) attention output over its 16 key blocks {128*(2u+p)} and the
  partial softmax row-sums. Host merges: O = (O_0 + O_1) / (rs_0 + rs_1).
  exp() without per-row max subtraction (scaled scores are in [-8, 8] for
  randn inputs; exp stays well inside fp32 range).

v3: q-projection work is additionally split between the two cores of a pair:
  core p computes qT only for query chunk-pairs {2i+p} (i=0..3, 512 cols
  each, from a host-prepared xq_own input), then a pairwise AllGather
  exchanges the halves through DRAM while phase B (kT/V build) runs.
  Attention chunks are processed in order [11..0, 15, 14, 13, 12] so the
  gathered qT is first needed long after the collective completes, and the
  kernel ends on big chunks whose matmul stream hides the output drains.
  All matmul operands bf16 (FWL weight loads); PSUM accumulation fp32.
"""

import sys

sys.path.insert(0, "/opt/trn_rl_repo")

import numpy as np
import ml_dtypes
from contextlib import ExitStack

import concourse.tile as tile
from concourse import bacc, mybir
from concourse.bass_utils import run_bass_kernel_spmd

P = 128
D = 1024
T = 4096
B = 4
NDB = D // P  # 8 d-blocks
NCB = D // P  # 8 contraction blocks
NKB = 16  # key blocks per core (parity half of 32)
QC = 256  # query-chunk columns in phase C
NQC = T // QC  # 16
CH = 512  # streaming column chunk
NSLOT = 4  # q chunk-pairs computed locally per core
F32 = mybir.dt.float32
BF16 = mybir.dt.bfloat16
EXPSCALE = 1.0 / 32.0  # 1/sqrt(D)
EXP = mybir.ActivationFunctionType.Exp
GROUPS = [[0, 1], [2, 3], [4, 5], [6, 7]]
ORDER = list(range(11, -1, -1)) + [15, 14, 13, 12]

_CACHED_NC = None
_LAST_RES = None


def _build_program():
    nc = bacc.Bacc("TRN2", target_bir_lowering=False, debug=False, num_devices=8)

    xqo_d = nc.dram_tensor("xqo", [D, NSLOT * CH], BF16, kind="ExternalInput").ap()
    xTk_d = nc.dram_tensor("xTk", [D, T // 2], BF16, kind="ExternalInput").ap()
    wq_d = nc.dram_tensor("WqT", [D, D], BF16, kind="ExternalInput").ap()
    wk_d = nc.dram_tensor("WkT", [D, D], BF16, kind="ExternalInput").ap()
    wv_d = nc.dram_tensor("WvT", [D, D], BF16, kind="ExternalInput").ap()
    mask_d = nc.dram_tensor("mask", [P, QC], F32, kind="ExternalInput").ap()
    ones2_d = nc.dram_tensor("ones2", [P, 2], BF16, kind="ExternalInput").ap()
    o_d = nc.dram_tensor("O", [T, D], BF16, kind="ExternalOutput").ap()
    rs_d = nc.dram_tensor("rs", [T, 1], F32, kind="ExternalOutput").ap()
    qown_d = nc.dram_tensor("qown", [D, NSLOT * CH], BF16).ap()  # internal
    qall_d = nc.dram_tensor("qall", [2 * D, NSLOT * CH], BF16).ap()  # internal

    xqo_r = xqo_d.rearrange("(a p) c -> p a c", p=P)  # [128, 8, 2048]
    xTk_r = xTk_d.rearrange("(a p) t -> p a t", p=P)  # [128, 8, 2048]
    wq_r = wq_d.rearrange("(a p) d -> p a d", p=P)  # [128, 8, 1024]
    wk_r = wk_d.rearrange("(a p) d -> p a d", p=P)
    wv_r = wv_d.rearrange("(a p) d -> p a d", p=P)
    qown_r = qown_d.rearrange("(a p) c -> p a c", p=P)  # [128, 8, 2048]
    qall_r = qall_d.rearrange("(r a p) c -> p (r a) c", p=P, r=2)  # [128, 16, 2048]

    with tile.TileContext(nc) as tc, ExitStack() as ctx:
        res = ctx.enter_context(tc.tile_pool(name="res", bufs=1))
        xkp = ctx.enter_context(tc.tile_pool(name="xkp", bufs=2))
        xqp = ctx.enter_context(tc.tile_pool(name="xqp", bufs=2))
        qop = ctx.enter_context(tc.tile_pool(name="qop", bufs=2))
        qtp = ctx.enter_context(tc.tile_pool(name="qtp", bufs=4))
        pp = ctx.enter_context(tc.tile_pool(name="pp", bufs=4))
        stg = ctx.enter_context(tc.tile_pool(name="stg", bufs=6))
        psum = ctx.enter_context(tc.tile_pool(name="psum", bufs=1, space="PSUM"))

        wq_res = res.tile([P, NCB, D], BF16, tag="wq")
        wk_res = res.tile([P, NCB, D], BF16, tag="wk")
        wv_res = res.tile([P, NCB, D], BF16, tag="wv")
        kt_t = res.tile([P, NDB, T // 2], BF16, tag="kt")  # [128, 8, 2048]
        v_t = res.tile([P, NKB, D + 2], BF16, tag="vt")  # [128, 16, 1026]
        mask_t = res.tile([P, QC], F32, tag="mask")

        # ---- head DMAs, critical path first: wq block0 + xq slot0 feed the
        # ---- very first q-projection matmuls; then just-in-time streaming.
        nc.sync.dma_start(wq_res[:, :, 0:P], wq_r[:, :, 0:P])
        xqs = {}
        for s in range(2):
            xqs[s] = xqp.tile([P, NCB, CH], BF16, tag="xq", name=f"xq{s}")
            nc.sync.dma_start(xqs[s][:], xqo_r[:, :, s * CH : (s + 1) * CH])
        nc.sync.dma_start(wq_res[:, :, P:D], wq_r[:, :, P:D])
        nc.gpsimd.dma_start(mask_t[:], mask_d[:])
        for kb in range(NKB):
            nc.gpsimd.dma_start(v_t[:, kb, D : D + 2], ones2_d[:])

        # ---------------- Phase A': local q-projection + exchange ------------
        for s in range(NSLOT):
            xq = xqs.pop(s)
            qo = qop.tile([P, NDB, CH], BF16, tag="qo", name=f"qo{s}")
            for db in range(NDB):
                ps = psum.tile([P, CH], F32, tag=f"b{6 + db % 2}", name=f"qp{s}_{db}")
                for cb in range(NCB):
                    nc.tensor.matmul(
                        ps[:],
                        wq_res[:, cb, db * P : (db + 1) * P],
                        xq[:, cb, :],
                        start=(cb == 0),
                        stop=(cb == NCB - 1),
                    )
                nc.vector.tensor_copy(qo[:, db, :], ps[:])
            nc.gpsimd.dma_start(qown_r[:, :, s * CH : (s + 1) * CH], qo[:])
            if s + 2 < NSLOT:  # prefetch with one-slot lead (pool bufs=2)
                xqs[s + 2] = xqp.tile([P, NCB, CH], BF16, tag="xq", name=f"xq{s + 2}")
                nc.sync.dma_start(
                    xqs[s + 2][:], xqo_r[:, :, (s + 2) * CH : (s + 3) * CH]
                )
            if s == 0:  # stream phase B's first stationary behind the xq slots
                nc.sync.dma_start(wk_res[:, :, 0:P], wk_r[:, :, 0:P])
        nc.gpsimd.collective_compute(
            "AllGather",
            mybir.AluOpType.bypass,
            replica_groups=GROUPS,
            ins=[qown_d[:]],
            outs=[qall_d[:]],
        )

        # ---------------- Phase B: kT + V (resident, bf16) -------------------
        xks = {}

        def xk_dma(g):
            xks[g] = xkp.tile([P, NCB, CH], BF16, tag="xk", name=f"xk{g}")
            nc.sync.dma_start(xks[g][:], xTk_r[:, :, g * CH : (g + 1) * CH])

        xk_dma(0)
        nc.sync.dma_start(wk_res[:, :, P:D], wk_r[:, :, P:D])
        nc.sync.dma_start(wv_res[:], wv_r[:])
        for g in range(4):  # groups of 4 key blocks (512 cols of xTk)
            if g + 1 < 4:
                xk_dma(g + 1)
            xk = xks.pop(g)
            for db in range(NDB):
                ps = psum.tile([P, CH], F32, tag=f"b{6 + db % 2}")
                for cb in range(NCB):
                    nc.tensor.matmul(
                        ps[:],
                        wk_res[:, cb, db * P : (db + 1) * P],
                        xk[:, cb, :],
                        start=(cb == 0),
                        stop=(cb == NCB - 1),
                    )
                nc.vector.tensor_copy(kt_t[:, db, g * CH : (g + 1) * CH], ps[:])
            for i in range(4):
                kb = 4 * g + i
                for h in range(2):
                    ps = psum.tile([P, CH], F32, tag=f"b{(2 * i + h) % 4}")
                    for cb in range(NCB):
                        nc.tensor.matmul(
                            ps[:],
                            xk[:, cb, i * P : (i + 1) * P],
                            wv_res[:, cb, h * CH : (h + 1) * CH],
                            start=(cb == 0),
                            stop=(cb == NCB - 1),
                        )
                    nc.vector.tensor_copy(v_t[:, kb, h * CH : (h + 1) * CH], ps[:])

        # ---------------- Phase C: attention (software-pipelined) -------------
        qt_tiles = {}

        def qt_dma(c):
            k = c // 2
            t = qtp.tile([P, NDB, QC], BF16, tag="qt", name=f"qt{c}")
            col = (k // 2) * CH + (c % 2) * QC
            nc.sync.dma_start(
                t[:], qall_r[:, (k % 2) * NDB : (k % 2 + 1) * NDB, col : col + QC]
            )
            qt_tiles[c] = t

        qt_dma(ORDER[0])
        qt_dma(ORDER[1])

        prev = None  # (acc dict, j) pending drain
        for oi, j in enumerate(ORDER):
            if oi + 2 < NQC:
                qt_dma(ORDER[oi + 2])
            qt = qt_tiles.pop(j)
            acc = {}
            for sub in range(2):
                for c in range(3):
                    shape = [P, 2] if c == 2 else [P, 512]
                    acc[sub, c] = psum.tile(
                        shape, F32, tag=f"b{sub * 3 + c}", name=f"acc{j}_{sub}_{c}"
                    )

            def av(u, pt_t, first, last):
                for sub in range(2):
                    lhs = pt_t[:, sub * P : (sub + 1) * P]
                    nc.tensor.matmul(
                        acc[sub, 0][:], lhs, v_t[:, u, 0:512],
                        start=first, stop=last, skip_group_check=True,
                    )
                    nc.tensor.matmul(
                        acc[sub, 1][:], lhs, v_t[:, u, 512:1024],
                        start=first, stop=last, skip_group_check=True,
                    )
                    nc.tensor.matmul(
                        acc[sub, 2][:], lhs, v_t[:, u, D : D + 2],
                        start=first, stop=last, skip_group_check=True,
                    )

            def drain(d_acc, d_j):
                for sub in range(2):
                    row = d_j * QC + sub * P
                    ot0 = stg.tile([P, 512], BF16, tag="stage", name=f"ot0_{d_j}_{sub}")
                    nc.vector.tensor_copy(ot0[:], d_acc[sub, 0][:])
                    ot1 = stg.tile([P, 512], BF16, tag="stage", name=f"ot1_{d_j}_{sub}")
                    nc.vector.tensor_copy(ot1[:], d_acc[sub, 1][:])
                    rt = stg.tile([P, 1], F32, tag="rt", name=f"rt{d_j}_{sub}")
                    nc.scalar.copy(rt[:], d_acc[sub, 2][:, 0:1])
                    nc.gpsimd.dma_start(o_d[row : row + P, 0:512], ot0[:])
                    nc.gpsimd.dma_start(o_d[row : row + P, 512:1024], ot1[:])
                    nc.gpsimd.dma_start(rs_d[row : row + P, :], rt[:])

            pts = {}
            for u in range(j + 1):
                st = psum.tile([P, QC], F32, tag=f"b{6 + u % 2}", name=f"st{j}_{u}")
                for db in range(NDB):
                    nc.tensor.matmul(
                        st[:],
                        kt_t[:, db, u * P : (u + 1) * P],
                        qt[:, db, :],
                        start=(db == 0),
                        stop=(db == NDB - 1),
                    )
                if u == j:
                    nc.vector.tensor_add(st[:], st[:], mask_t[:])
                pt = pp.tile([P, QC], BF16, tag="pt", name=f"pt{j}_{u}")
                nc.scalar.activation(pt[:], st[:], EXP, scale=EXPSCALE)
                pts[u] = pt
                if u == 2 and prev is not None:
                    drain(*prev)
                    prev = None
                if u >= 2:
                    av(u - 2, pts.pop(u - 2), first=(u == 2), last=False)
            if prev is not None:  # j in {0, 1}
                drain(*prev)
                prev = None
            if j >= 1:
                av(j - 1, pts.pop(j - 1), first=(j == 1), last=False)
            av(j, pts.pop(j), first=(j == 0), last=True)
            prev = (acc, j)
        drain_acc, drain_j = prev
        for sub in range(2):
            row = drain_j * QC + sub * P
            ot0 = stg.tile([P, 512], BF16, tag="stage", name=f"fot0_{sub}")
            nc.vector.tensor_copy(ot0[:], drain_acc[sub, 0][:])
            ot1 = stg.tile([P, 512], BF16, tag="stage", name=f"fot1_{sub}")
            nc.vector.tensor_copy(ot1[:], drain_acc[sub, 1][:])
            rt = stg.tile([P, 1], F32, tag="rt", name=f"frt{sub}")
            nc.scalar.copy(rt[:], drain_acc[sub, 2][:, 0:1])
            q = nc.sync if sub == 0 else nc.gpsimd
            q.dma_start(o_d[row : row + P, 0:512], ot0[:])
            q.dma_start(o_d[row : row + P, 512:1024], ot1[:])
            q.dma_start(rs_d[row : row + P, :], rt[:])

    nc.finalize()
    return nc


def _get_program():
    global _CACHED_NC
    if _CACHED_NC is None:
        _CACHED_NC = _build_program()
    return _CACHED_NC


def _masks():
    neg = np.float32(-1e30)
    tri = np.where(np.triu(np.ones((P, P), dtype=bool)), np.float32(0), neg)
    keep = np.zeros((P, P), dtype=np.float32)
    drop = np.full((P, P), neg, dtype=np.float32)
    return (
        np.ascontiguousarray(np.concatenate([tri, keep], axis=1)),  # even core
        np.ascontiguousarray(np.concatenate([drop, tri], axis=1)),  # odd core
    )


def kernel(x, Wq, Wk, Wv):
    out, _ = _run(x, Wq, Wk, Wv, trace=False)
    return out


def _run(x, Wq, Wk, Wv, trace=False, keep_res=False):
    bf = ml_dtypes.bfloat16
    x = np.asarray(x, dtype=np.float32)
    WqT = np.ascontiguousarray(np.asarray(Wq, dtype=np.float32).T.astype(bf))
    WkT = np.ascontiguousarray(np.asarray(Wk, dtype=np.float32).T.astype(bf))
    WvT = np.ascontiguousarray(np.asarray(Wv, dtype=np.float32).T.astype(bf))
    m_even, m_odd = _masks()
    ones2 = np.ascontiguousarray(
        np.repeat(np.array([[1.0, 0.0]], dtype=np.float32), P, axis=0).astype(bf)
    )

    nc = _get_program()
    in_maps = []
    for core in range(8):
        b, p = core // 2, core % 2
        xT = np.ascontiguousarray(x[b].T.astype(bf))  # [D, T]
        xTk = np.ascontiguousarray(
            xT.reshape(D, T // P, P)[:, p::2, :].reshape(D, T // 2)
        )
        xqo = np.ascontiguousarray(
            np.concatenate(
                [xT[:, CH * (2 * i + p) : CH * (2 * i + p + 1)] for i in range(NSLOT)],
                axis=1,
            )
        )
        in_maps.append(
            {
                "xqo": xqo,
                "xTk": xTk,
                "WqT": WqT,
                "WkT": WkT,
                "WvT": WvT,
                "mask": m_even if p == 0 else m_odd,
                "ones2": ones2,
            }
        )

    res = run_bass_kernel_spmd(nc, in_maps, core_ids=list(range(8)), trace=trace)
    if keep_res:
        global _LAST_RES
        _LAST_RES = res
    out = np.empty((B, T, D), dtype=np.float32)
    for b in range(B):
        O0 = res.results[2 * b]["O"].astype(np.float32)
        rs0 = res.results[2 * b]["rs"]
        O1 = res.results[2 * b + 1]["O"].astype(np.float32)
        rs1 = res.results[2 * b + 1]["rs"]
        out[b] = (O0 + O1) / (rs0 + rs1)
    return out, res.exec_time_ns


# revision 8
# speedup vs baseline: 1.2115x; 1.2115x over previous
"""Causal single-head attention (B=4, T=4096, D=1024) on 8 trn2 NeuronCores.

Sharding: 2 cores per batch element, split by key-block PARITY (flash-style):
  core = 2*b + p ; p in {0,1}
  Each core computes, for ALL 4096 queries of batch b, the partial
  (unnormalized) attention output over its 16 key blocks {128*(2u+p)} and the
  partial softmax row-sums. Host merges: O = (O_0 + O_1) / (rs_0 + rs_1).
  exp() without per-row max subtraction (scaled scores are in [-8, 8] for
  randn inputs; exp stays well inside fp32 range).

v4: q-projection split between the two cores of a pair: core p computes qT
  for 4 of the 8 query chunk-pairs (host-prepared xqo input), exchanged via
  4 small pairwise AllGathers fired as each 512-col slot completes, ordered
  so the earliest-needed pairs exchange first. Attention chunks run in order
  [11..0, 15, 14, 13, 12]: gathered qT is first needed well after AllGather
  #0 completes, and the kernel ends on big chunks that hide output drains.
  qT reads ride the gpsimd queue (they depend on the collectives, which must
  stay on gpsimd); output drains ride sync so they never queue behind a
  collective. All matmul operands bf16 (FWL); PSUM accumulation fp32.

  slot s of core p holds query chunk-pair SLOT_PAIRS[s] + p; qall_s gathers
  the even core's slot (rank 0) then the odd core's (rank 1).
"""

import sys

sys.path.insert(0, "/opt/trn_rl_repo")

import numpy as np
import ml_dtypes
from contextlib import ExitStack

import concourse.tile as tile
from concourse import bacc, mybir
from concourse.bass_utils import run_bass_kernel_spmd

P = 128
D = 1024
T = 4096
B = 4
NDB = D // P  # 8 d-blocks
NCB = D // P  # 8 contraction blocks
NKB = 16  # key blocks per core (parity half of 32)
QC = 256  # query-chunk columns in phase C
NQC = T // QC  # 16
CH = 512  # streaming column chunk
NSLOT = 4  # q chunk-pairs computed locally per core
SLOT_PAIRS = [4, 2, 0, 6]  # + core parity; exchange order = first-needed first
SLOT_OF_PAIR = {4: 0, 5: 0, 2: 1, 3: 1, 0: 2, 1: 2, 6: 3, 7: 3}
F32 = mybir.dt.float32
BF16 = mybir.dt.bfloat16
EXPSCALE = 1.0 / 32.0  # 1/sqrt(D)
EXP = mybir.ActivationFunctionType.Exp
GROUPS = [[0, 1], [2, 3], [4, 5], [6, 7]]
ORDER = list(range(11, -1, -1)) + [15, 14, 13, 12]

_CACHED_NC = None
_LAST_RES = None


def _build_program():
    nc = bacc.Bacc("TRN2", target_bir_lowering=False, debug=False, num_devices=8)

    xqo_d = nc.dram_tensor("xqo", [D, NSLOT * CH], BF16, kind="ExternalInput").ap()
    xTk_d = nc.dram_tensor("xTk", [D, T // 2], BF16, kind="ExternalInput").ap()
    wq_d = nc.dram_tensor("WqT", [D, D], BF16, kind="ExternalInput").ap()
    wk_d = nc.dram_tensor("WkT", [D, D], BF16, kind="ExternalInput").ap()
    wv_d = nc.dram_tensor("WvT", [D, D], BF16, kind="ExternalInput").ap()
    mask_d = nc.dram_tensor("mask", [P, QC], F32, kind="ExternalInput").ap()
    ones2_d = nc.dram_tensor("ones2", [P, 2], BF16, kind="ExternalInput").ap()
    o_d = nc.dram_tensor("O", [T, D], BF16, kind="ExternalOutput").ap()
    rs_d = nc.dram_tensor("rs", [T, 1], F32, kind="ExternalOutput").ap()
    qown = [nc.dram_tensor(f"qown{s}", [D, CH], BF16).ap() for s in range(NSLOT)]
    qall = [nc.dram_tensor(f"qall{s}", [2 * D, CH], BF16).ap() for s in range(NSLOT)]

    xqo_r = xqo_d.rearrange("(a p) c -> p a c", p=P)  # [128, 8, 2048]
    xTk_r = xTk_d.rearrange("(a p) t -> p a t", p=P)  # [128, 8, 2048]
    wq_r = wq_d.rearrange("(a p) d -> p a d", p=P)  # [128, 8, 1024]
    wk_r = wk_d.rearrange("(a p) d -> p a d", p=P)
    wv_r = wv_d.rearrange("(a p) d -> p a d", p=P)
    qown_r = [q.rearrange("(a p) c -> p a c", p=P) for q in qown]  # [128, 8, 512]
    qall_r = [
        q.rearrange("(r a p) c -> p (r a) c", p=P, r=2) for q in qall
    ]  # [128, 16, 512]

    with tile.TileContext(nc) as tc, ExitStack() as ctx:
        res = ctx.enter_context(tc.tile_pool(name="res", bufs=1))
        xkp = ctx.enter_context(tc.tile_pool(name="xkp", bufs=2))
        xqp = ctx.enter_context(tc.tile_pool(name="xqp", bufs=2))
        qop = ctx.enter_context(tc.tile_pool(name="qop", bufs=2))
        qtp = ctx.enter_context(tc.tile_pool(name="qtp", bufs=6))
        pp = ctx.enter_context(tc.tile_pool(name="pp", bufs=4))
        stg = ctx.enter_context(tc.tile_pool(name="stg", bufs=6))
        psum = ctx.enter_context(tc.tile_pool(name="psum", bufs=1, space="PSUM"))

        wq_res = res.tile([P, NCB, D], BF16, tag="wq")
        wk_res = res.tile([P, NCB, D], BF16, tag="wk")
        wv_res = res.tile([P, NCB, D], BF16, tag="wv")
        kt_t = res.tile([P, NDB, T // 2], BF16, tag="kt")  # [128, 8, 2048]
        v_t = res.tile([P, NKB, D + 2], BF16, tag="vt")  # [128, 16, 1026]
        mask_t = res.tile([P, QC], F32, tag="mask")

        # ---- head DMAs, ordered for the first q-projection matmuls ----------
        nc.sync.dma_start(wq_res[:, :, 0 : 2 * P], wq_r[:, :, 0 : 2 * P])
        xqs = {}

        def xq_dma(s):
            xqs[s] = xqp.tile([P, NCB, CH], BF16, tag="xq", name=f"xq{s}")
            nc.sync.dma_start(xqs[s][:], xqo_r[:, :, s * CH : (s + 1) * CH])

        xq_dma(0)
        nc.sync.dma_start(wq_res[:, :, 2 * P : D], wq_r[:, :, 2 * P : D])
        xq_dma(1)
        nc.sync.dma_start(wk_res[:, :, 0:P], wk_r[:, :, 0:P])
        nc.gpsimd.dma_start(mask_t[:], mask_d[:])
        for kb in range(NKB):
            nc.gpsimd.dma_start(v_t[:, kb, D : D + 2], ones2_d[:])

        # ---------------- Phase A': local q-projection + exchange ------------
        for s in range(NSLOT):
            xq = xqs.pop(s)
            qo = qop.tile([P, NDB, CH], BF16, tag="qo", name=f"qo{s}")
            for db in range(NDB):
                ps = psum.tile([P, CH], F32, tag=f"b{6 + db % 2}", name=f"qp{s}_{db}")
                for cb in range(NCB):
                    nc.tensor.matmul(
                        ps[:],
                        wq_res[:, cb, db * P : (db + 1) * P],
                        xq[:, cb, :],
                        start=(cb == 0),
                        stop=(cb == NCB - 1),
                    )
                nc.vector.tensor_copy(qo[:, db, :], ps[:])
            nc.gpsimd.dma_start(qown_r[s][:], qo[:])
            nc.gpsimd.collective_compute(
                "AllGather",
                mybir.AluOpType.bypass,
                replica_groups=GROUPS,
                ins=[qown[s][:]],
                outs=[qall[s][:]],
            )
            if s + 2 < NSLOT:  # prefetch with one-slot lead (pool bufs=2)
                xq_dma(s + 2)

        # ---------------- Phase B: kT + V (resident, bf16) -------------------
        xks = {}

        def xk_dma(g):
            xks[g] = xkp.tile([P, NCB, CH], BF16, tag="xk", name=f"xk{g}")
            nc.sync.dma_start(xks[g][:], xTk_r[:, :, g * CH : (g + 1) * CH])

        xk_dma(0)
        nc.sync.dma_start(wk_res[:, :, P:D], wk_r[:, :, P:D])
        nc.sync.dma_start(wv_res[:], wv_r[:])
        for g in range(4):  # groups of 4 key blocks (512 cols of xTk)
            if g + 1 < 4:
                xk_dma(g + 1)
            xk = xks.pop(g)
            for db in range(NDB):
                ps = psum.tile([P, CH], F32, tag=f"b{6 + db % 2}")
                for cb in range(NCB):
                    nc.tensor.matmul(
                        ps[:],
                        wk_res[:, cb, db * P : (db + 1) * P],
                        xk[:, cb, :],
                        start=(cb == 0),
                        stop=(cb == NCB - 1),
                    )
                nc.vector.tensor_copy(kt_t[:, db, g * CH : (g + 1) * CH], ps[:])
            for i in range(4):
                kb = 4 * g + i
                for h in range(2):
                    ps = psum.tile([P, CH], F32, tag=f"b{(2 * i + h) % 4}")
                    for cb in range(NCB):
                        nc.tensor.matmul(
                            ps[:],
                            xk[:, cb, i * P : (i + 1) * P],
                            wv_res[:, cb, h * CH : (h + 1) * CH],
                            start=(cb == 0),
                            stop=(cb == NCB - 1),
                        )
                    nc.vector.tensor_copy(v_t[:, kb, h * CH : (h + 1) * CH], ps[:])

        # ---------------- Phase C: attention (software-pipelined) -------------
        qt_tiles = {}

        def qt_dma(c):
            k = c // 2
            s = SLOT_OF_PAIR[k]
            t = qtp.tile([P, NDB, QC], BF16, tag="qt", name=f"qt{c}")
            col = (c % 2) * QC
            nc.gpsimd.dma_start(
                t[:], qall_r[s][:, (k % 2) * NDB : (k % 2 + 1) * NDB, col : col + QC]
            )
            qt_tiles[c] = t

        for c in ORDER[:4]:
            qt_dma(c)

        prev = None  # (acc dict, j) pending drain
        for oi, j in enumerate(ORDER):
            if oi + 4 < NQC:
                qt_dma(ORDER[oi + 4])
            qt = qt_tiles.pop(j)
            acc = {}
            for sub in range(2):
                for c in range(3):
                    shape = [P, 2] if c == 2 else [P, 512]
                    acc[sub, c] = psum.tile(
                        shape, F32, tag=f"b{sub * 3 + c}", name=f"acc{j}_{sub}_{c}"
                    )

            def av(u, pt_t, first, last):
                for sub in range(2):
                    lhs = pt_t[:, sub * P : (sub + 1) * P]
                    nc.tensor.matmul(
                        acc[sub, 0][:], lhs, v_t[:, u, 0:512],
                        start=first, stop=last, skip_group_check=True,
                    )
                    nc.tensor.matmul(
                        acc[sub, 1][:], lhs, v_t[:, u, 512:1024],
                        start=first, stop=last, skip_group_check=True,
                    )
                    nc.tensor.matmul(
                        acc[sub, 2][:], lhs, v_t[:, u, D : D + 2],
                        start=first, stop=last, skip_group_check=True,
                    )

            def drain(d_acc, d_j):
                for sub in range(2):
                    row = d_j * QC + sub * P
                    ot0 = stg.tile([P, 512], BF16, tag="stage", name=f"ot0_{d_j}_{sub}")
                    nc.vector.tensor_copy(ot0[:], d_acc[sub, 0][:])
                    ot1 = stg.tile([P, 512], BF16, tag="stage", name=f"ot1_{d_j}_{sub}")
                    nc.vector.tensor_copy(ot1[:], d_acc[sub, 1][:])
                    rt = stg.tile([P, 1], F32, tag="rt", name=f"rt{d_j}_{sub}")
                    nc.scalar.copy(rt[:], d_acc[sub, 2][:, 0:1])
                    nc.sync.dma_start(o_d[row : row + P, 0:512], ot0[:])
                    nc.sync.dma_start(o_d[row : row + P, 512:1024], ot1[:])
                    nc.sync.dma_start(rs_d[row : row + P, :], rt[:])

            pts = {}
            for u in range(j + 1):
                st = psum.tile([P, QC], F32, tag=f"b{6 + u % 2}", name=f"st{j}_{u}")
                for db in range(NDB):
                    nc.tensor.matmul(
                        st[:],
                        kt_t[:, db, u * P : (u + 1) * P],
                        qt[:, db, :],
                        start=(db == 0),
                        stop=(db == NDB - 1),
                    )
                if u == j:
                    nc.vector.tensor_add(st[:], st[:], mask_t[:])
                pt = pp.tile([P, QC], BF16, tag="pt", name=f"pt{j}_{u}")
                nc.scalar.activation(pt[:], st[:], EXP, scale=EXPSCALE)
                pts[u] = pt
                if u == 2 and prev is not None:
                    drain(*prev)
                    prev = None
                if u >= 2:
                    av(u - 2, pts.pop(u - 2), first=(u == 2), last=False)
            if prev is not None:  # j in {0, 1}
                drain(*prev)
                prev = None
            if j >= 1:
                av(j - 1, pts.pop(j - 1), first=(j == 1), last=False)
            av(j, pts.pop(j), first=(j == 0), last=True)
            prev = (acc, j)
        drain_acc, drain_j = prev
        for sub in range(2):
            row = drain_j * QC + sub * P
            ot0 = stg.tile([P, 512], BF16, tag="stage", name=f"fot0_{sub}")
            nc.vector.tensor_copy(ot0[:], drain_acc[sub, 0][:])
            ot1 = stg.tile([P, 512], BF16, tag="stage", name=f"fot1_{sub}")
            nc.vector.tensor_copy(ot1[:], drain_acc[sub, 1][:])
            rt = stg.tile([P, 1], F32, tag="rt", name=f"frt{sub}")
            nc.scalar.copy(rt[:], drain_acc[sub, 2][:, 0:1])
            q = nc.sync if sub == 0 else nc.gpsimd
            q.dma_start(o_d[row : row + P, 0:512], ot0[:])
            q.dma_start(o_d[row : row + P, 512:1024], ot1[:])
            q.dma_start(rs_d[row : row + P, :], rt[:])

    nc.finalize()
    return nc


def _get_program():
    global _CACHED_NC
    if _CACHED_NC is None:
        _CACHED_NC = _build_program()
    return _CACHED_NC


def _masks():
    neg = np.float32(-1e30)
    tri = np.where(np.triu(np.ones((P, P), dtype=bool)), np.float32(0), neg)
    keep = np.zeros((P, P), dtype=np.float32)
    drop = np.full((P, P), neg, dtype=np.float32)
    return (
        np.ascontiguousarray(np.concatenate([tri, keep], axis=1)),  # even core
        np.ascontiguousarray(np.concatenate([drop, tri], axis=1)),  # odd core
    )


def kernel(x, Wq, Wk, Wv):
    out, _ = _run(x, Wq, Wk, Wv, trace=False)
    return out


def _run(x, Wq, Wk, Wv, trace=False, keep_res=False):
    bf = ml_dtypes.bfloat16
    x = np.asarray(x, dtype=np.float32)
    WqT = np.ascontiguousarray(np.asarray(Wq, dtype=np.float32).T.astype(bf))
    WkT = np.ascontiguousarray(np.asarray(Wk, dtype=np.float32).T.astype(bf))
    WvT = np.ascontiguousarray(np.asarray(Wv, dtype=np.float32).T.astype(bf))
    m_even, m_odd = _masks()
    ones2 = np.ascontiguousarray(
        np.repeat(np.array([[1.0, 0.0]], dtype=np.float32), P, axis=0).astype(bf)
    )

    nc = _get_program()
    in_maps = []
    for core in range(8):
        b, p = core // 2, core % 2
        xT = np.ascontiguousarray(x[b].T.astype(bf))  # [D, T]
        xTk = np.ascontiguousarray(
            xT.reshape(D, T // P, P)[:, p::2, :].reshape(D, T // 2)
        )
        xqo = np.ascontiguousarray(
            np.concatenate(
                [
                    xT[:, CH * (k + p) : CH * (k + p + 1)]
                    for k in SLOT_PAIRS
                ],
                axis=1,
            )
        )
        in_maps.append(
            {
                "xqo": xqo,
                "xTk": xTk,
                "WqT": WqT,
                "WkT": WkT,
                "WvT": WvT,
                "mask": m_even if p == 0 else m_odd,
                "ones2": ones2,
            }
        )

    res = run_bass_kernel_spmd(nc, in_maps, core_ids=list(range(8)), trace=trace)
    if keep_res:
        global _LAST_RES
        _LAST_RES = res
    out = np.empty((B, T, D), dtype=np.float32)
    for b in range(B):
        O0 = res.results[2 * b]["O"].astype(np.float32)
        rs0 = res.results[2 * b]["rs"]
        O1 = res.results[2 * b + 1]["O"].astype(np.float32)
        rs1 = res.results[2 * b + 1]["rs"]
        out[b] = (O0 + O1) / (rs0 + rs1)
    return out, res.exec_time_ns


# revision 16
# speedup vs baseline: 1.2842x; 1.0601x over previous
"""Causal single-head attention (B=4, T=4096, D=1024) on 8 trn2 NeuronCores.

Sharding: 2 cores per batch element, split by key-block PARITY (flash-style):
  core = 2*b + p ; p in {0,1}
  Each core computes, for ALL 4096 queries of batch b, the partial
  (unnormalized) attention output over its 16 key blocks {128*(2u+p)} and the
  partial softmax row-sums. Host merges: O = (O_0 + O_1) / (rs_0 + rs_1).
  exp() without per-row max subtraction (scaled scores are in [-8, 8] for
  randn inputs; exp stays well inside fp32 range).

v4: q-projection split between the two cores of a pair: core p computes qT
  for 4 of the 8 query chunk-pairs (host-prepared xqo input), exchanged via
  4 small pairwise AllGathers fired as each 512-col slot completes, ordered
  so the earliest-needed pairs exchange first. Attention chunks run in order
  [11..0, 15, 14, 13, 12]: gathered qT is first needed well after AllGather
  #0 completes, and the kernel ends on big chunks that hide output drains.
  qT reads ride the gpsimd queue (they depend on the collectives, which must
  stay on gpsimd); output drains ride sync so they never queue behind a
  collective. All matmul operands bf16 (FWL); PSUM accumulation fp32.

  slot s of core p holds query chunk-pair SLOT_PAIRS[s] + p; qall_s gathers
  the even core's slot (rank 0) then the odd core's (rank 1).
"""

import sys

sys.path.insert(0, "/opt/trn_rl_repo")

import numpy as np
import ml_dtypes
from contextlib import ExitStack

import concourse.tile as tile
from concourse import bacc, mybir
from concourse.bass_utils import run_bass_kernel_spmd

P = 128
D = 1024
T = 4096
B = 4
NDB = D // P  # 8 d-blocks
NCB = D // P  # 8 contraction blocks
NKB = 16  # key blocks per core (parity half of 32)
QC = 256  # query-chunk columns in phase C
NQC = T // QC  # 16
CH = 512  # streaming column chunk
NSLOT = 4  # q chunk-pairs computed locally per core
SLOT_PAIRS = [4, 2, 0, 6]  # + core parity; exchange order = first-needed first
SLOT_OF_PAIR = {4: 0, 5: 0, 2: 1, 3: 1, 0: 2, 1: 2, 6: 3, 7: 3}
F32 = mybir.dt.float32
BF16 = mybir.dt.bfloat16
EXPSCALE = 1.0 / 32.0  # 1/sqrt(D)
EXP = mybir.ActivationFunctionType.Exp
GROUPS = [[0, 1], [2, 3], [4, 5], [6, 7]]
ORDER = list(range(11, -1, -1)) + [15, 14, 13, 12]

_CACHED_NC = None
_LAST_RES = None


def _build_program():
    nc = bacc.Bacc("TRN2", target_bir_lowering=False, debug=False, num_devices=8)

    xqo_d = nc.dram_tensor("xqo", [D, NSLOT * CH], BF16, kind="ExternalInput").ap()
    xTk_d = nc.dram_tensor("xTk", [D, T // 2], BF16, kind="ExternalInput").ap()
    wq_d = nc.dram_tensor("WqT", [D, D], BF16, kind="ExternalInput").ap()
    wk_d = nc.dram_tensor("WkT", [D, D], BF16, kind="ExternalInput").ap()
    wv_d = nc.dram_tensor("WvT", [D, D], BF16, kind="ExternalInput").ap()
    mask_d = nc.dram_tensor("mask", [P, QC], F32, kind="ExternalInput").ap()
    ones2_d = nc.dram_tensor("ones2", [P, 2], BF16, kind="ExternalInput").ap()
    o_d = nc.dram_tensor("O", [T, D], BF16, kind="ExternalOutput").ap()
    rs_d = nc.dram_tensor("rs", [T, 1], F32, kind="ExternalOutput").ap()
    qown = [nc.dram_tensor(f"qown{s}", [D, CH], BF16).ap() for s in range(NSLOT)]
    qall = [nc.dram_tensor(f"qall{s}", [2 * D, CH], BF16).ap() for s in range(NSLOT)]

    xqo_r = xqo_d.rearrange("(a p) c -> p a c", p=P)  # [128, 8, 2048]
    xTk_r = xTk_d.rearrange("(a p) t -> p a t", p=P)  # [128, 8, 2048]
    wq_r = wq_d.rearrange("(a p) d -> p a d", p=P)  # [128, 8, 1024]
    wk_r = wk_d.rearrange("(a p) d -> p a d", p=P)
    wv_r = wv_d.rearrange("(a p) d -> p a d", p=P)
    qown_r = [q.rearrange("(a p) c -> p a c", p=P) for q in qown]  # [128, 8, 512]
    qall_r = [
        q.rearrange("(r a p) c -> p (r a) c", p=P, r=2) for q in qall
    ]  # [128, 16, 512]

    with tile.TileContext(nc) as tc, ExitStack() as ctx:
        res = ctx.enter_context(tc.tile_pool(name="res", bufs=1))
        xkp = ctx.enter_context(tc.tile_pool(name="xkp", bufs=2))
        xqp = ctx.enter_context(tc.tile_pool(name="xqp", bufs=4))
        qop = ctx.enter_context(tc.tile_pool(name="qop", bufs=2))
        qtp = ctx.enter_context(tc.tile_pool(name="qtp", bufs=4))
        pp = ctx.enter_context(tc.tile_pool(name="pp", bufs=4))
        stg = ctx.enter_context(tc.tile_pool(name="stg", bufs=4))
        psum = ctx.enter_context(tc.tile_pool(name="psum", bufs=1, space="PSUM"))

        wq_res = res.tile([P, NCB, D], BF16, tag="wq")
        wk_res = res.tile([P, NCB, D], BF16, tag="wk")
        wv_res = res.tile([P, NCB, D], BF16, tag="wv")
        kt_t = res.tile([P, NDB, T // 2], BF16, tag="kt")  # [128, 8, 2048]
        v_t = res.tile([P, NKB, D + 2], BF16, tag="vt")  # [128, 16, 1026]
        mask_t = res.tile([P, QC], F32, tag="mask")

        # ---- head DMAs, ordered for the first q-projection matmuls ----------
        nc.sync.dma_start(wq_res[:, :, 0 : 2 * P], wq_r[:, :, 0 : 2 * P])
        xqs = {}

        def xq_dma(s):
            xqs[s] = xqp.tile([P, NCB, CH], BF16, tag="xq", name=f"xq{s}")
            nc.sync.dma_start(xqs[s][:], xqo_r[:, :, s * CH : (s + 1) * CH])

        xq_dma(0)
        nc.sync.dma_start(wq_res[:, :, 2 * P : 4 * P], wq_r[:, :, 2 * P : 4 * P])
        xq_dma(1)
        nc.sync.dma_start(wq_res[:, :, 4 * P : D], wq_r[:, :, 4 * P : D])
        xq_dma(2)
        xq_dma(3)
        nc.sync.dma_start(wk_res[:, :, 0:P], wk_r[:, :, 0:P])
        nc.gpsimd.dma_start(mask_t[:], mask_d[:])
        for kb in range(NKB):
            nc.gpsimd.dma_start(v_t[:, kb, D : D + 2], ones2_d[:])

        # ---------------- Phase A': local q-projection + exchange ------------
        for s in range(NSLOT):
            xq = xqs.pop(s)
            qo = qop.tile([P, NDB, CH], BF16, tag="qo", name=f"qo{s}")
            for db in range(NDB):
                ps = psum.tile([P, CH], F32, tag=f"b{6 + db % 2}", name=f"qp{s}_{db}")
                for cb in range(NCB):
                    nc.tensor.matmul(
                        ps[:],
                        wq_res[:, cb, db * P : (db + 1) * P],
                        xq[:, cb, :],
                        start=(cb == 0),
                        stop=(cb == NCB - 1),
                    )
                nc.vector.tensor_copy(qo[:, db, :], ps[:])
            nc.gpsimd.dma_start(qown_r[s][:], qo[:])
            nc.gpsimd.collective_compute(
                "AllGather",
                mybir.AluOpType.bypass,
                replica_groups=GROUPS,
                ins=[qown[s][:]],
                outs=[qall[s][:]],
            )

        # ---------------- Phase B: kT + V (resident, bf16) -------------------
        xks = {}

        def xk_dma(g):
            xks[g] = xkp.tile([P, NCB, CH], BF16, tag="xk", name=f"xk{g}")
            nc.sync.dma_start(xks[g][:], xTk_r[:, :, g * CH : (g + 1) * CH])

        xk_dma(0)
        nc.sync.dma_start(wk_res[:, :, P:D], wk_r[:, :, P:D])
        nc.sync.dma_start(wv_res[:], wv_r[:])
        for g in range(4):  # groups of 4 key blocks (512 cols of xTk)
            if g + 1 < 4:
                xk_dma(g + 1)
            xk = xks.pop(g)
            for db in range(NDB):
                ps = psum.tile([P, CH], F32, tag=f"b{6 + db % 2}")
                for cb in range(NCB):
                    nc.tensor.matmul(
                        ps[:],
                        wk_res[:, cb, db * P : (db + 1) * P],
                        xk[:, cb, :],
                        start=(cb == 0),
                        stop=(cb == NCB - 1),
                    )
                nc.vector.tensor_copy(kt_t[:, db, g * CH : (g + 1) * CH], ps[:])
            for i in range(4):
                kb = 4 * g + i
                for h in range(2):
                    ps = psum.tile([P, CH], F32, tag=f"b{(2 * i + h) % 4}")
                    for cb in range(NCB):
                        nc.tensor.matmul(
                            ps[:],
                            xk[:, cb, i * P : (i + 1) * P],
                            wv_res[:, cb, h * CH : (h + 1) * CH],
                            start=(cb == 0),
                            stop=(cb == NCB - 1),
                        )
                    nc.vector.tensor_copy(v_t[:, kb, h * CH : (h + 1) * CH], ps[:])

        # ---------------- Phase C: attention (software-pipelined) -------------
        qt_tiles = {}

        def qt_dma(c):
            k = c // 2
            s = SLOT_OF_PAIR[k]
            t = qtp.tile([P, NDB, QC], BF16, tag="qt", name=f"qt{c}")
            col = (c % 2) * QC
            nc.gpsimd.dma_start(
                t[:], qall_r[s][:, (k % 2) * NDB : (k % 2 + 1) * NDB, col : col + QC]
            )
            qt_tiles[c] = t

        for c in ORDER[:3]:
            qt_dma(c)

        prev = None  # (acc dict, j) pending drain
        for oi, j in enumerate(ORDER):
            if oi + 3 < NQC:
                qt_dma(ORDER[oi + 3])
            qt = qt_tiles.pop(j)
            acc = {}
            for sub in range(2):
                for c in range(3):
                    shape = [P, 2] if c == 2 else [P, 512]
                    acc[sub, c] = psum.tile(
                        shape, F32, tag=f"b{sub * 3 + c}", name=f"acc{j}_{sub}_{c}"
                    )

            def av(u, pt_t, first, last):
                for sub in range(2):
                    lhs = pt_t[:, sub * P : (sub + 1) * P]
                    nc.tensor.matmul(
                        acc[sub, 0][:], lhs, v_t[:, u, 0:512],
                        start=first, stop=last, skip_group_check=True,
                    )
                    nc.tensor.matmul(
                        acc[sub, 1][:], lhs, v_t[:, u, 512:1024],
                        start=first, stop=last, skip_group_check=True,
                    )
                    nc.tensor.matmul(
                        acc[sub, 2][:], lhs, v_t[:, u, D : D + 2],
                        start=first, stop=last, skip_group_check=True,
                    )

            def drain(d_acc, d_j):
                # vector takes the low d-half, scalar the high half: the two
                # engines read different PSUM banks in parallel, halving the
                # copy latency that gates the next chunk's first AV matmuls.
                for sub in range(2):
                    row = d_j * QC + sub * P
                    ot0 = stg.tile([P, 512], BF16, tag="stage", name=f"ot0_{d_j}_{sub}")
                    nc.vector.tensor_copy(ot0[:], d_acc[sub, 0][:])
                    ot1 = stg.tile([P, 512], BF16, tag="stg2", name=f"ot1_{d_j}_{sub}")
                    nc.scalar.copy(ot1[:], d_acc[sub, 1][:])
                    rt = stg.tile([P, 1], F32, tag="rt", name=f"rt{d_j}_{sub}")
                    nc.scalar.copy(rt[:], d_acc[sub, 2][:, 0:1])
                    nc.sync.dma_start(o_d[row : row + P, 0:512], ot0[:])
                    nc.sync.dma_start(o_d[row : row + P, 512:1024], ot1[:])
                    nc.sync.dma_start(rs_d[row : row + P, :], rt[:])

            if prev is not None:  # drain at chunk start: copies overlap scores
                drain(*prev)
                prev = None

            pts = {}
            for u in range(j + 1):
                st = psum.tile([P, QC], F32, tag=f"b{6 + u % 2}", name=f"st{j}_{u}")
                for db in range(NDB):
                    nc.tensor.matmul(
                        st[:],
                        kt_t[:, db, u * P : (u + 1) * P],
                        qt[:, db, :],
                        start=(db == 0),
                        stop=(db == NDB - 1),
                    )
                if u == j:
                    nc.vector.tensor_add(st[:], st[:], mask_t[:])
                pt = pp.tile([P, QC], BF16, tag="pt", name=f"pt{j}_{u}")
                nc.scalar.activation(pt[:], st[:], EXP, scale=EXPSCALE)
                pts[u] = pt
                if u >= 2:
                    av(u - 2, pts.pop(u - 2), first=(u == 2), last=False)
            if j >= 1:
                av(j - 1, pts.pop(j - 1), first=(j == 1), last=False)
            av(j, pts.pop(j), first=(j == 0), last=True)
            prev = (acc, j)
        drain_acc, drain_j = prev
        for sub in range(2):
            row = drain_j * QC + sub * P
            ot0 = stg.tile([P, 512], BF16, tag="stage", name=f"fot0_{sub}")
            nc.vector.tensor_copy(ot0[:], drain_acc[sub, 0][:])
            ot1 = stg.tile([P, 512], BF16, tag="stg2", name=f"fot1_{sub}")
            nc.scalar.copy(ot1[:], drain_acc[sub, 1][:])
            rt = stg.tile([P, 1], F32, tag="rt", name=f"frt{sub}")
            nc.scalar.copy(rt[:], drain_acc[sub, 2][:, 0:1])
            nc.sync.dma_start(o_d[row : row + P, 0:512], ot0[:])
            nc.sync.dma_start(o_d[row : row + P, 512:1024], ot1[:])
            nc.sync.dma_start(rs_d[row : row + P, :], rt[:])

    nc.finalize()
    return nc


def _get_program():
    global _CACHED_NC
    if _CACHED_NC is None:
        _CACHED_NC = _build_program()
    return _CACHED_NC


def _masks():
    neg = np.float32(-1e30)
    tri = np.where(np.triu(np.ones((P, P), dtype=bool)), np.float32(0), neg)
    keep = np.zeros((P, P), dtype=np.float32)
    drop = np.full((P, P), neg, dtype=np.float32)
    return (
        np.ascontiguousarray(np.concatenate([tri, keep], axis=1)),  # even core
        np.ascontiguousarray(np.concatenate([drop, tri], axis=1)),  # odd core
    )


def kernel(x, Wq, Wk, Wv):
    out, _ = _run(x, Wq, Wk, Wv, trace=False)
    return out


def _run(x, Wq, Wk, Wv, trace=False, keep_res=False):
    bf = ml_dtypes.bfloat16
    x = np.asarray(x, dtype=np.float32)
    WqT = np.ascontiguousarray(np.asarray(Wq, dtype=np.float32).T.astype(bf))
    WkT = np.ascontiguousarray(np.asarray(Wk, dtype=np.float32).T.astype(bf))
    WvT = np.ascontiguousarray(np.asarray(Wv, dtype=np.float32).T.astype(bf))
    m_even, m_odd = _masks()
    ones2 = np.ascontiguousarray(
        np.repeat(np.array([[1.0, 0.0]], dtype=np.float32), P, axis=0).astype(bf)
    )

    nc = _get_program()
    in_maps = []
    for core in range(8):
        b, p = core // 2, core % 2
        xT = np.ascontiguousarray(x[b].T.astype(bf))  # [D, T]
        xTk = np.ascontiguousarray(
            xT.reshape(D, T // P, P)[:, p::2, :].reshape(D, T // 2)
        )
        xqo = np.ascontiguousarray(
            np.concatenate(
                [
                    xT[:, CH * (k + p) : CH * (k + p + 1)]
                    for k in SLOT_PAIRS
                ],
                axis=1,
            )
        )
        in_maps.append(
            {
                "xqo": xqo,
                "xTk": xTk,
                "WqT": WqT,
                "WkT": WkT,
                "WvT": WvT,
                "mask": m_even if p == 0 else m_odd,
                "ones2": ones2,
            }
        )

    res = run_bass_kernel_spmd(nc, in_maps, core_ids=list(range(8)), trace=trace)
    if keep_res:
        global _LAST_RES
        _LAST_RES = res
    out = np.empty((B, T, D), dtype=np.float32)
    for b in range(B):
        O0 = res.results[2 * b]["O"].astype(np.float32)
        rs0 = res.results[2 * b]["rs"]
        O1 = res.results[2 * b + 1]["O"].astype(np.float32)
        rs1 = res.results[2 * b + 1]["rs"]
        out[b] = (O0 + O1) / (rs0 + rs1)
    return out, res.exec_time_ns
